# revision 15
# baseline (speedup 1.0000x reference)
"""Trainium2 Bass kernel for the sparse-attention module (nn_EAP_v1).

Distribution: 8 cores = batch(4) x H-halves(2), 1-row halo. Each core works
on a zero-padded 65x130 grid (grid row i = global row start-1+i, col j =
global col j-1); x=0 at pads makes q/k/v exactly 0 there (1x1 conv, no
bias), matching the reference's zero-padded unfold/conv.

Per-core pipeline (positions live on SBUF partitions):
  phase 1: kv GEMM  x[c,pos] (stationary) @ Wkv -> psum[pos, 512];
           ACT evicts to bf16 grid with per-position sum / sum-of-squares
           accumulators -> LN stats u, rstd per grid position.
  phase 2A (per 128-position out tile): the whole q path (1x1 conv +
           depthwise 3x3 s2 conv) is one fused im2col GEMM over x with
           host-folded weights W3[(cin,tap), o]; extra GEMM columns yield
           q-LN mean and the per-head constants Craw/Araw. q-LN + logits
           fold LN of k into per-position stats:
             logits[t,h] = rstd_k_t * sum_c(qt * k_t) + A - ur_k_t * C
  phase 2B: batched softmax over the 9 taps for all 16 tiles.
  phase 2C: V = sum_t (a*rstd_v_t) * v_t ; S = sum_t a*ur_v_t ;
           PE-transpose [V | S | 1] and project with LN-v scale/bias and
           the -S@W_h correction folded into the GEMM (K=9 block).
Taps are fetched with SBUF->SBUF DMAs (partition-remapping gathers); tap
stats go through a small DRAM roundtrip for clean flat addressing.

Self-contained: hardcodes all shapes; needs only numpy/ml_dtypes/concourse.
"""

import numpy as np
import ml_dtypes

import concourse.bass as bass
import concourse.bacc as bacc
import concourse.mybir as mybir
import concourse.tile as tile
from concourse.bass_utils import run_bass_kernel_spmd

FP32 = mybir.dt.float32
BF16 = mybir.dt.bfloat16
AF = mybir.ActivationFunctionType
ALU = mybir.AluOpType
AX = mybir.AxisListType

B = 4
HEAD_DIM = 32
NH = 8
SCALE = HEAD_DIM ** -0.5
EPS = 1e-6

NR, NC_ = 65, 130          # padded grid rows/cols per core
G = NR * NC_               # 8450
NGT = 67                   # 65 row tiles (j=0..127) + edge cols j=128, j=129
NOT_ = 16                  # out tiles; tile T = out rows {2T, 2T+1}, p = 2s+rr
N_LOCAL = 2048
W3N = 256 + 8 + 8 + 1      # qd | Craw | Araw | sum_q

TAPS = [(t // 3, t % 3) for t in range(9)]


def build_module():
    nc = bacc.Bacc()

    x_d = nc.dram_tensor("x", [2, 128, G], BF16, kind="ExternalInput")
    wkv_d = nc.dram_tensor("wkv", [2, 128, 512], BF16, kind="ExternalInput")
    w3_d = nc.dram_tensor("w3", [18, 128, W3N], BF16, kind="ExternalInput")
    wpt_d = nc.dram_tensor("wpt", [2, 128, 256], BF16, kind="ExternalInput")
    wsb_d = nc.dram_tensor("wsb", [9, 256], BF16, kind="ExternalInput")
    g12_d = nc.dram_tensor("g12", [128, 2, 256], BF16, kind="ExternalInput")
    kk_d = nc.dram_tensor("kk", [128, 4, 8], FP32, kind="ExternalInput")
    ident_d = nc.dram_tensor("ident", [128, 128], BF16, kind="ExternalInput")
    out_d = nc.dram_tensor("out", [N_LOCAL, 256], FP32, kind="ExternalOutput")

    with tile.TileContext(nc) as tc:
        with (
            tc.tile_pool(name="const", bufs=1) as const,
            tc.tile_pool(name="grid", bufs=1) as gridp,
            tc.tile_pool(name="work", bufs=3) as work,
            tc.tile_pool(name="taps", bufs=2) as tapsp,
            tc.tile_pool(name="small", bufs=3) as small,
            tc.tile_pool(name="dramp", bufs=1, space="DRAM") as dramp,
            tc.tile_pool(name="ps_kv", bufs=2, space="PSUM") as ps_kv,
            tc.tile_pool(name="ps_qd", bufs=2, space="PSUM") as ps_qd,
            tc.tile_pool(name="ps_tr", bufs=2, space="PSUM") as ps_tr,
            tc.tile_pool(name="ps_pj", bufs=2, space="PSUM") as ps_pj,
        ):
            # ---------- load constants ----------
            x_sb = const.tile([128, 2, G], BF16, tag="x_sb")
            nc.sync.dma_start(out=x_sb[:, 0, :], in_=x_d[0])
            nc.sync.dma_start(out=x_sb[:, 1, :], in_=x_d[1])
            wkv_sb = const.tile([128, 2, 512], BF16, tag="wkv_sb")
            nc.sync.dma_start(out=wkv_sb[:], in_=wkv_d.rearrange("t p n -> p t n"))
            w3_sb = const.tile([128, 18, W3N], BF16, tag="w3_sb")
            nc.sync.dma_start(out=w3_sb[:], in_=w3_d.rearrange("t p n -> p t n"))
            wpt_sb = const.tile([128, 2, 256], BF16, tag="wpt_sb")
            nc.sync.dma_start(out=wpt_sb[:], in_=wpt_d.rearrange("t p n -> p t n"))
            wsb_sb = const.tile([9, 256], BF16, tag="wsb_sb")
            nc.sync.dma_start(out=wsb_sb[:], in_=wsb_d[:])
            g12_sb = const.tile([128, 2, 256], BF16, tag="g12_sb")
            nc.sync.dma_start(out=g12_sb[:], in_=g12_d[:])
            kk_sb = const.tile([128, 4, 8], FP32, tag="kk_sb")
            nc.sync.dma_start(out=kk_sb[:], in_=kk_d[:])
            ident_sb = const.tile([128, 128], BF16, tag="ident_sb")
            nc.sync.dma_start(out=ident_sb[:], in_=ident_d[:])
            eps_sb = const.tile([128, 1], FP32, tag="eps_sb")
            nc.vector.memset(eps_sb[:], float(EPS))
            ident32_sb = const.tile([128, 128], FP32, tag="ident32_sb")
            nc.vector.tensor_copy(out=ident32_sb[:], in_=ident_sb[:])

            kvg = gridp.tile([128, NGT, 512], BF16, tag="kvg")
            sumk = gridp.tile([128, NGT], FP32, tag="sumk")
            sumv = gridp.tile([128, NGT], FP32, tag="sumv")
            sqk = gridp.tile([128, NGT], FP32, tag="sqk")
            sqv = gridp.tile([128, NGT], FP32, tag="sqv")
            # only the never-written garbage region (edge slots, partitions
            # 65..127) needs defined values for the stats-finish ops
            nc.vector.memset(sumk[64:128, 65:67], 0.0)
            nc.vector.memset(sumv[64:128, 65:67], 0.0)
            nc.vector.memset(sqk[64:128, 65:67], 0.0)
            nc.vector.memset(sqv[64:128, 65:67], 0.0)

            x_g = [x_sb[:, ct, :].rearrange("p (i j) -> p i j", j=NC_)
                   for ct in range(2)]

            kvd = dramp.tile([G, 512], BF16, tag="kvd")
            statd = dramp.tile([G, 4], FP32, tag="statd")
            kvd_ji = kvd[:].rearrange("(i j) c -> j i c", j=NC_)
            kvd_ij = kvd[:].rearrange("(i j) c -> i j c", j=NC_)

            # ---------- phase 1: kv GEMM + evict + stats ----------
            for gt in range(NGT):
                if gt < 65:
                    P = 128
                    lhs = [x_g[ct][:, gt, 0:128] for ct in range(2)]
                else:
                    P = 65
                    j = 128 + (gt - 65)
                    lhs = [x_g[ct][:, :, j] for ct in range(2)]
                pkv = ps_kv.tile([128, 512], FP32, tag="pkv")
                for ct in range(2):
                    nc.tensor.matmul(pkv[:P], lhs[ct], wkv_sb[:, ct, :],
                                     start=(ct == 0), stop=(ct == 1))
                nc.scalar.activation(out=kvg[:P, gt, 0:256], in_=pkv[:P, 0:256],
                                     func=AF.Copy, accum_out=sumk[:P, gt:gt + 1])
                nc.scalar.activation(out=kvg[:P, gt, 256:512], in_=pkv[:P, 256:512],
                                     func=AF.Copy, accum_out=sumv[:P, gt:gt + 1])
                sq = work.tile([128, 512], BF16, tag="sqscratch")
                nc.scalar.activation(out=sq[:P, 0:256], in_=pkv[:P, 0:256],
                                     func=AF.Square, accum_out=sqk[:P, gt:gt + 1])
                nc.scalar.activation(out=sq[:P, 256:512], in_=pkv[:P, 256:512],
                                     func=AF.Square, accum_out=sqv[:P, gt:gt + 1])
                if gt < 65:
                    nc.sync.dma_start(out=kvd_ji[0:128, gt, :], in_=kvg[:, gt, :])
                else:
                    nc.sync.dma_start(out=kvd_ij[:, 63 + gt, :],
                                      in_=kvg[0:65, gt, :])

            # ---------- stats finish:  u=sum/256, var=sq/256-u^2 ----------

            def finish_stats(nm, sum_t, sq_t):
                u = gridp.tile([128, NGT], FP32, tag=f"u_{nm}")
                rstd = gridp.tile([128, NGT], FP32, tag=f"rstd_{nm}")
                ur = gridp.tile([128, NGT], FP32, tag=f"ur_{nm}")
                nc.scalar.mul(out=u[:], in_=sum_t[:], mul=1.0 / 256.0)
                nc.vector.tensor_mul(out=sum_t[:], in0=u[:], in1=u[:])
                nc.scalar.mul(out=rstd[:], in_=sq_t[:], mul=1.0 / 256.0)
                nc.vector.tensor_sub(out=rstd[:], in0=rstd[:], in1=sum_t[:])
                nc.scalar.activation(out=rstd[:], in_=rstd[:], func=AF.Sqrt,
                                     bias=eps_sb[:])
                nc.vector.reciprocal(out=rstd[:], in_=rstd[:])
                nc.vector.tensor_mul(out=ur[:], in0=u[:], in1=rstd[:])
                return rstd, ur

            rstdk, urk = finish_stats("k", sumk, sqk)
            rstdv, urv = finish_stats("v", sumv, sqv)

            # transpose stats to i-major, interleave kinds, write contiguously
            statTI = gridp.tile([128, 128, 4], FP32, tag="statTI")
            for k4, st in enumerate((rstdk, urk, rstdv, urv)):
                ptr = ps_qd.tile([128, W3N], FP32, tag="pqd")
                nc.tensor.transpose(ptr[:67, 0:128], st[:], ident32_sb[:])
                nc.scalar.copy(out=statTI[0:67, :, k4], in_=ptr[:67, 0:128])
            statd_ij4 = statd[:].rearrange("(i j) k -> i j k", j=NC_)
            nc.sync.dma_start(out=statd_ij4[0:65, 0:128, :],
                              in_=statTI[0:65, :, :])
            for e in range(2):
                nc.sync.dma_start(out=statd_ij4[:, 128 + e, :],
                                  in_=statTI[65 + e:66 + e, 0:65, :])

            # ---------- gather tap stats: stat_big [128, T, t, kind] ----------
            stat_big = const.tile([128, NOT_, 9, 4], FP32, tag="stat_big")
            sd0 = statd[:]
            for T in range(NOT_):
                for t, (di, dj) in enumerate(TAPS):
                    src = bass.AP(
                        tensor=sd0.tensor,
                        offset=sd0.offset + ((4 * T + di) * NC_ + dj) * 4,
                        ap=[[2 * NC_ * 4, 2], [2 * 4, 64], [1, 4]],
                    )
                    nc.sync.dma_start(out=stat_big[:, T, t, :], in_=src)

            G_all = const.tile([128, NOT_, 9, 8], FP32, tag="G_all")
            e_all = const.tile([128, NOT_, 9, 8], FP32, tag="e_all")
            at_all = const.tile([128, NOT_, 9, 8], BF16, tag="at_all")
            s_all = const.tile([128, NOT_, 8], FP32, tag="s_all")
            A_all = const.tile([128, NOT_, 8], FP32, tag="A_all")
            C_all = const.tile([128, NOT_, 8], FP32, tag="C_all")

            kv0 = kvd[:]

            def tap_dma(dst, T, half):
                co = 256 * half
                for t, (di, dj) in enumerate(TAPS):
                    for rr in range(2):
                        row = 4 * T + di + 2 * rr
                        p0 = 64 * rr
                        if dj == 1:
                            src = bass.AP(
                                tensor=kv0.tensor,
                                offset=kv0.offset + (row * NC_ + 1) * 512 + co,
                                ap=[[2 * 512, 64], [1, 256]],
                            )
                            nc.sync.dma_start(out=dst[p0:p0 + 64, t, :], in_=src)
                        elif dj == 0:
                            nc.sync.dma_start(out=dst[p0:p0 + 64, t, :],
                                              in_=kvg[0::2, row, co:co + 256])
                        else:
                            nc.sync.dma_start(out=dst[p0:p0 + 63, t, :],
                                              in_=kvg[2:128:2, row, co:co + 256])
                            nc.sync.dma_start(
                                out=dst[p0 + 63:p0 + 64, t, :],
                                in_=kvg[row:row + 1, 65, co:co + 256])

            # ---------- phase 2A: fused conv GEMM + q side + G ----------
            for T in range(NOT_):
                pqd = ps_qd.tile([128, W3N], FP32, tag="pqd")
                for rr in range(2):
                    mm = 0
                    for t, (di, dj) in enumerate(TAPS):
                        for ct in range(2):
                            lhs = x_g[ct][:, 4 * T + di + 2 * rr,
                                          dj:dj + 128:2]  # [128, 64]
                            nc.tensor.matmul(
                                pqd[64 * rr:64 * (rr + 1)], lhs,
                                w3_sb[:, 2 * t + ct, :],
                                start=(mm == 0), stop=(mm == 17),
                                tile_position=(0, 64 * rr))
                            mm += 1
                sqq = small.tile([128, 1], FP32, tag="sqq")
                sqs = work.tile([128, 256], BF16, tag="sqscratch2")
                nc.scalar.activation(out=sqs[:], in_=pqd[:, 0:256],
                                     func=AF.Square, accum_out=sqq[:])
                uq = small.tile([128, 1], FP32, tag="uq")
                nc.scalar.mul(out=uq[:], in_=pqd[:, 272:273], mul=1.0 / 256.0)
                var = small.tile([128, 1], FP32, tag="var")
                nc.scalar.mul(out=var[:], in_=sqq[:], mul=1.0 / 256.0)
                u2 = small.tile([128, 1], FP32, tag="u2")
                nc.vector.tensor_mul(out=u2[:], in0=uq[:], in1=uq[:])
                nc.vector.tensor_sub(out=var[:], in0=var[:], in1=u2[:])
                nc.scalar.activation(out=var[:], in_=var[:], func=AF.Sqrt,
                                     bias=eps_sb[:])
                rstdq = small.tile([128, 1], FP32, tag="rstdq")
                nc.vector.reciprocal(out=rstdq[:], in_=var[:])
                urq = small.tile([128, 1], FP32, tag="urq")
                nc.vector.tensor_mul(out=urq[:], in0=uq[:], in1=rstdq[:])
                nbias = small.tile([128, 1], FP32, tag="nbias")
                nc.scalar.mul(out=nbias[:], in_=urq[:], mul=-1.0)
                yhat = work.tile([128, 256], BF16, tag="yhat")
                nc.scalar.activation(out=yhat[:], in_=pqd[:, 0:256], func=AF.Copy,
                                     bias=0.0, scale=rstdq[:])
                nc.vector.tensor_scalar_add(out=yhat[:], in0=yhat[:],
                                            scalar1=nbias[:])
                qt = work.tile([128, 256], BF16, tag="qt")
                nc.vector.tensor_mul(out=qt[:], in0=yhat[:], in1=g12_sb[:, 0, :])
                nc.vector.tensor_add(out=qt[:], in0=qt[:], in1=g12_sb[:, 1, :])
                for dst, col0, km, ka in ((C_all, 256, 0, 1), (A_all, 264, 2, 3)):
                    tmp = small.tile([128, 8], FP32, tag="ca_tmp")
                    nc.vector.tensor_scalar_mul(out=tmp[:],
                                                in0=pqd[:, col0:col0 + 8],
                                                scalar1=rstdq[:])
                    tmp2 = small.tile([128, 8], FP32, tag="ca_tmp2")
                    nc.vector.tensor_scalar_mul(out=tmp2[:], in0=kk_sb[:, km, :],
                                                scalar1=urq[:])
                    nc.vector.tensor_sub(out=tmp[:], in0=tmp[:], in1=tmp2[:])
                    nc.vector.tensor_add(out=dst[:, T, :], in0=tmp[:],
                                         in1=kk_sb[:, ka, :])
                kt = tapsp.tile([128, 9, 256], BF16, tag="kt")
                tap_dma(kt, T, 0)
                prod = work.tile([128, 9, 256], BF16, tag="prod")
                qt_b = qt[:, None, :].to_broadcast([128, 9, 256])
                nc.vector.tensor_mul(out=prod[:], in0=kt[:], in1=qt_b)
                nc.vector.tensor_reduce(
                    out=G_all[:, T, :, :],
                    in_=prod[:].rearrange("p t (h c) -> p t h c", c=32),
                    axis=AX.X, op=ALU.add)

            # ---------- phase 2B: batched logits + softmax ----------
            logit = const.tile([128, NOT_, 9, 8], FP32, tag="logit")
            sh4 = [128, NOT_, 9, 8]
            def stat_b(kind):
                return stat_big[:, :, :, kind, None].to_broadcast(sh4)

            rk_b = stat_b(0)
            uk_b = stat_b(1)
            C_b = C_all[:, :, None, :].to_broadcast(sh4)
            A_b = A_all[:, :, None, :].to_broadcast(sh4)
            nc.vector.tensor_mul(out=logit[:], in0=G_all[:], in1=rk_b)
            nc.vector.tensor_add(out=logit[:], in0=logit[:], in1=A_b)
            nc.vector.tensor_mul(out=G_all[:], in0=uk_b, in1=C_b)
            nc.vector.tensor_sub(out=logit[:], in0=logit[:], in1=G_all[:])
            mx = const.tile([128, NOT_, 8], FP32, tag="mx")
            nc.vector.tensor_reduce(out=mx[:],
                                    in_=logit[:].rearrange("p T t h -> p T h t"),
                                    axis=AX.X, op=ALU.max)
            mx_b = mx[:, :, None, :].to_broadcast(sh4)
            nc.vector.tensor_sub(out=logit[:], in0=logit[:], in1=mx_b)
            nc.scalar.activation(out=e_all[:], in_=logit[:], func=AF.Exp)
            se = const.tile([128, NOT_, 8], FP32, tag="se")
            nc.vector.tensor_reduce(out=se[:],
                                    in_=e_all[:].rearrange("p T t h -> p T h t"),
                                    axis=AX.X, op=ALU.add)
            nc.vector.reciprocal(out=se[:], in_=se[:])
            rs_b = se[:, :, None, :].to_broadcast(sh4)
            rv_b = stat_b(2)
            uv_b = stat_b(3)
            w1 = const.tile([128, NOT_, 9, 8], FP32, tag="w1")
            nc.vector.tensor_mul(out=w1[:], in0=rs_b, in1=rv_b)
            nc.vector.tensor_mul(out=w1[:], in0=w1[:], in1=e_all[:])
            nc.vector.tensor_copy(out=at_all[:], in_=w1[:])
            nc.vector.tensor_mul(out=w1[:], in0=e_all[:], in1=uv_b)
            nc.vector.tensor_reduce(out=s_all[:],
                                    in_=w1[:].rearrange("p T t h -> p T h t"),
                                    axis=AX.X, op=ALU.add)
            nc.vector.tensor_mul(out=s_all[:], in0=s_all[:], in1=se[:])

            # ---------- phase 2C: v sum + transpose + proj ----------
            ov = out_d.rearrange("(T r s) o -> T r s o", T=NOT_, r=2, s=64)
            for T in range(NOT_):
                vt = tapsp.tile([128, 9, 256], BF16, tag="vt")
                tap_dma(vt, T, 1)
                vp = work.tile([128, 9, 256], BF16, tag="vp")
                at_b = at_all[:, T, :, :, None].to_broadcast([128, 9, 8, 32])
                nc.vector.tensor_tensor(
                    out=vp[:].rearrange("p t (h c) -> p t h c", c=32),
                    in0=vt[:].rearrange("p t (h c) -> p t h c", c=32),
                    in1=at_b, op=ALU.mult)
                ae = work.tile([128, 265], BF16, tag="ae")
                nc.vector.tensor_add(out=vp[:, 0:4, :], in0=vp[:, 0:4, :],
                                     in1=vp[:, 4:8, :])
                nc.vector.tensor_add(out=vp[:, 0:2, :], in0=vp[:, 0:2, :],
                                     in1=vp[:, 2:4, :])
                nc.vector.tensor_add(out=vp[:, 0, :], in0=vp[:, 0, :],
                                     in1=vp[:, 1, :])
                nc.vector.tensor_add(out=ae[:, 0:256], in0=vp[:, 0, :],
                                     in1=vp[:, 8, :])
                nc.vector.tensor_copy(out=ae[:, 256:264], in_=s_all[:, T, :])
                nc.vector.memset(ae[:, 264:265], 1.0)
                atT = work.tile([128, 2, 128], BF16, tag="atT")
                asT = work.tile([9, 128], BF16, tag="asT")
                for blk in range(2):
                    ptr = ps_tr.tile([128, 128], BF16, tag="ptr")
                    nc.tensor.transpose(ptr[:], ae[:, 128 * blk:128 * (blk + 1)],
                                        ident_sb[:])
                    nc.scalar.copy(out=atT[:, blk, :], in_=ptr[:])
                ptr = ps_tr.tile([128, 128], BF16, tag="ptr")
                nc.tensor.transpose(ptr[:9], ae[:, 256:265], ident_sb[:])
                nc.scalar.copy(out=asT[:], in_=ptr[:9])
                ppj = ps_pj.tile([128, 256], FP32, tag="ppj")
                for ct in range(2):
                    nc.tensor.matmul(ppj[:], atT[:, ct, :], wpt_sb[:, ct, :],
                                     start=(ct == 0), stop=False)
                nc.tensor.matmul(ppj[:], asT[:], wsb_sb[:], start=False, stop=True)
                oev = work.tile([128, 256], FP32, tag="oev")
                nc.scalar.copy(out=oev[:], in_=ppj[:])
                nc.sync.dma_start(out=ov[T], in_=oev[:])

    nc.finalize()
    return nc


# ----------------------------------------------------------------------------
# Host-side preparation
# ----------------------------------------------------------------------------

def _prep_shared(w_qkv, w_dwq, ln_q_w, ln_q_b, ln_k_w, ln_k_b, ln_v_w, ln_v_b,
                 w_proj, b_proj):
    f32 = np.float32
    bf = ml_dtypes.bfloat16
    w_qkv = w_qkv.astype(f32)
    wq, wk, wv = w_qkv[0:256], w_qkv[256:512], w_qkv[512:768]
    ln_q_w, ln_q_b = ln_q_w.astype(f32), ln_q_b.astype(f32)
    ln_k_w, ln_k_b = ln_k_w.astype(f32), ln_k_b.astype(f32)
    ln_v_w, ln_v_b = ln_v_w.astype(f32), ln_v_b.astype(f32)

    wkv = np.empty((2, 128, 512), f32)
    for ct in range(2):
        wkv[ct, :, 0:256] = wk[:, 128 * ct:128 * (ct + 1)].T
        wkv[ct, :, 256:512] = wv[:, 128 * ct:128 * (ct + 1)].T

    g1 = ln_q_w * ln_k_w * SCALE
    g2 = ln_q_b * ln_k_w * SCALE
    g3 = ln_q_w * ln_k_b * SCALE
    c4v = (ln_q_b * ln_k_b * SCALE).reshape(NH, HEAD_DIM).sum(1)
    K2 = g1.reshape(NH, HEAD_DIM).sum(1)
    K3 = g2.reshape(NH, HEAD_DIM).sum(1)
    K4 = g3.reshape(NH, HEAD_DIM).sum(1)

    dw = w_dwq.reshape(256, 9).astype(f32)
    hb = np.zeros((256, NH), f32)
    for h in range(NH):
        hb[32 * h:32 * (h + 1), h] = 1.0
    w3 = np.empty((18, 128, W3N), f32)
    for t in range(9):
        w3t = wq.T * dw[:, t][None, :]          # [cin, o]
        ext = np.concatenate([w3t, w3t @ (g1[:, None] * hb),
                              w3t @ (g3[:, None] * hb),
                              w3t.sum(1, keepdims=True)], axis=1)
        for ct in range(2):
            w3[2 * t + ct] = ext[128 * ct:128 * (ct + 1)]

    wp_f = w_proj.astype(f32) * ln_v_w[None, :]     # [o, c] * lnvw[c]
    wpt = np.empty((2, 128, 256), f32)
    for ct in range(2):
        wpt[ct] = wp_f[:, 128 * ct:128 * (ct + 1)].T
    Wh = wp_f.T.reshape(NH, HEAD_DIM, 256).sum(1)   # [h, o]
    bias_f = b_proj.astype(f32) + w_proj.astype(f32) @ ln_v_b
    wsb = np.concatenate([-Wh, bias_f[None, :]], axis=0)

    def rep(v):
        return np.broadcast_to(v[None], (128,) + v.shape).copy()

    g12 = np.stack([rep(g1), rep(g2)], axis=1)
    kk = np.stack([rep(K2), rep(K3), rep(K4), rep(c4v)], axis=1)

    return dict(
        wkv=wkv.astype(bf), w3=w3.astype(bf), wpt=wpt.astype(bf),
        wsb=wsb.astype(bf), g12=g12.astype(bf),
        kk=np.ascontiguousarray(kk, f32),
        ident=np.eye(128, dtype=f32).astype(bf),
    )


def _shard_x(x):
    bf = ml_dtypes.bfloat16
    shards = []
    for b in range(B):
        for half in range(2):
            r0 = half * 64 - 1
            xp = np.zeros((256, NR, NC_), np.float32)
            gr0, gr1 = max(r0, 0), min(r0 + NR, 128)
            xp[:, gr0 - r0:gr1 - r0, 1:129] = x[b, :, gr0:gr1, :]
            shards.append(xp.reshape(2, 128, G).astype(bf))
    return shards


_NC_CACHE = {}


def kernel(x, w_qkv, w_dwq, ln_q_w, ln_q_b, ln_k_w, ln_k_b, ln_v_w, ln_v_b,
           w_proj, b_proj):
    x = np.asarray(x, np.float32)
    shared = _prep_shared(np.asarray(w_qkv), np.asarray(w_dwq),
                          np.asarray(ln_q_w), np.asarray(ln_q_b),
                          np.asarray(ln_k_w), np.asarray(ln_k_b),
                          np.asarray(ln_v_w), np.asarray(ln_v_b),
                          np.asarray(w_proj), np.asarray(b_proj))
    shards = _shard_x(x)
    in_maps = [dict(shared, x=shards[i]) for i in range(8)]

    if "nc" not in _NC_CACHE:
        _NC_CACHE["nc"] = build_module()
    nc = _NC_CACHE["nc"]

    res = run_bass_kernel_spmd(nc, in_maps, list(range(8)))
    out = np.empty((B, 4096, 256), np.float32)
    for b in range(B):
        for half in range(2):
            core = b * 2 + half
            out[b, half * 2048:(half + 1) * 2048, :] = res.results[core]["out"]
    return (out, 64, 64)
